# revision 36
# baseline (speedup 1.0000x reference)
"""Multi-head attention (RMSNorm q/k, dense softmax) on 8 TRN2 NeuronCores.

Sharding: core c -> batch b = c//2, head-group g = c%2 (8 of 16 heads).
Each core computes a partial y.T = (Wo_g @ O_g.T) for its batch; the host
sums the two head-group partials per batch and transposes back.

Device dataflow (per core; layouts keep the contraction dim on SBUF
partitions; x and all weights are pre-transposed AND pre-converted to bf16
on the host so every DMA is contiguous and every matmul runs at full PE
rate; fp32 accumulation happens in PSUM):
  V    = x @ Wv_g.T              -> V_aug [t, 8, 65] bf16 (ones column yields
                                    softmax denominators during the PV matmul)
  Q.T  = Wq_g @ x.T, RMS-normed  -> bf16 [128 (2 heads x 64 dh), 2048 t]
  K.T  likewise
  S.T  = K.T-slices^T @ Q.T      -> PSUM [128 k-tok, 1024]
  P.T  = exp(S.T / 8)            -> bf16 SBUF (ScalarE; no max-subtraction:
                                    |logits| <= 8 because q,k are RMS-normed)
  O.T  = V_aug^T @ P.T           -> PSUM [65, q]; row 64 = sum(exp) = den
  O.T /= den   (reciprocal_approx_fast + GpSimd partition_broadcast)
  y.T  = Wo_g.T^T @ O.T          -> PSUM -> SBUF -> DRAM

Scheduling structure (the performance-critical part). A two-phase version
(all projections+RMS, then attention) ran ~455us: the projection phase left
ScalarE idle and the attention phase was paced by the exp stream while PE
had slack. PE's mandatory column-work is only ~341us, so this version runs
ONE fused instruction stream in which PE is never intentionally idle
(~418us measured):
- block order is qc-outer / hp-inner, so each token chunk's output
  projection spreads over the following 64-step sweep instead of cramming
  into the last head-pair's blocks;
- x moves as whole [128,2048] tiles (4KB rows sustain ~360GB/s vs ~200
  for 1KB rows) on the Sync DMA queue while wv/wk go out in parallel on
  the Scalar engine's DMA queue; the V projection runs dt-outer across
  all 8 PSUM banks so its first chain issues ~11us in, paced by the x
  stream;
- the head computes only what attention step 0 needs: V tiles 0-7,
  K(hp0), Q(hp0, chunk 0); the V projection's second half is emitted
  AFTER those chunks on the other four banks so the Tile scheduler fills
  the chunk chains' ScalarE-latency stalls with V matmuls;
- every remaining projection chunk and output-projection piece is a
  "filler" assigned to a specific attention step within its deadline
  (K(hp,c4) is first read at step 16*hp + 4*c4), ~4-5 chunks per sweep-0
  block, one Q chunk (for block B+2) per later block; RMSNorm ln/exp ride
  the exp activation table (the get_activation_tables edit below keeps
  ln+exp on ONE table set: no reloads), each chunk split a/b several
  steps apart so PE never waits on the ScalarE queue;
- attention itself is the flat 2-deep pipeline per step k: S.T(k),
  exp(k-1), PV(k-2), with a 6-deep P.T ring;
- softmax denominators ride the PV matmul via the V ones-column; the
  normalize runs off the PV critical path (copy frees the accumulator, the
  reciprocal+broadcast happen on DVE/GpSimd, the multiply is deferred into
  the next block and flushed at j==2, before that chunk's output
  projection reads O.T).
PSUM: st0+st1 [128,1024] (4 banks) + pv x2 (2) + qk-proj bank (1) +
outproj bank (1) = 8.

Measurement notes: (a) the attached trn2 devices alternate between two
clock states across runs (~x1.17 uniform duration ratio on every engine;
~418us fast-state vs ~495us slow-state for this kernel, ~455 vs ~530 for
the two-phase version) — compare timings only within a state, via matmul
mean duration (~390ns fast vs ~467ns slow for 512-col bf16);
(b) deprioritizing fillers via tc.high_priority(-3000) deadlocks on
hardware — don't; (c) without the get_activation_tables edit, LN and EXP
land on different activation tables and every interleaved RMS chunk costs
two ~1.5us ACT_TABLE_LOADs (+170us total).
"""

import numpy as np
import ml_dtypes

B, N, D, H, Dh = 4, 2048, 1024, 16, 64
HPC = 8  # heads per core
GD = HPC * Dh  # 512 out-dims per core per projection
EPS = float(np.finfo(np.float32).eps)
NT = N // 128  # 16 token tiles
DT = D // 128  # 8 contraction tiles over D
KT = GD // 128  # 4 contraction tiles over the head-dim group

_NC_CACHE = {}


def _build_nc():
    import concourse.tile as tile
    from concourse import bacc, mybir

    f32 = mybir.dt.float32
    bf16 = mybir.dt.bfloat16
    AF = mybir.ActivationFunctionType

    nc = bacc.Bacc(None, target_bir_lowering=False)

    # Ln and Exp both live in the 'natural_log_exp_and_others' activation
    # table set, but the table-load pass assigns each function its first
    # containing set, picking two different tables — every RMS ln/exp
    # interleaved into the attention exp stream would then cost two
    # ~1.5us ACT_TABLE_LOADs. Strip exp/ln from the other sets (names and
    # indices preserved) so both resolve to the combined set and the
    # whole kernel runs on one table.
    from concourse.hw_specs import get_activation_tables

    tabs = get_activation_tables(nc.m.arch)  # functools.cache'd dict
    for name, fns in tabs.items():
        if name != "natural_log_exp_and_others":
            fns.discard(mybir.ActivationFunctionType.Exp)
            fns.discard(mybir.ActivationFunctionType.Ln)

    xT_e = nc.declare_dram_parameter("xT", [D, N], bf16, isOutput=False)
    wqT_e = nc.declare_dram_parameter("wqT", [D, GD], bf16, isOutput=False)
    wkT_e = nc.declare_dram_parameter("wkT", [D, GD], bf16, isOutput=False)
    wvT_e = nc.declare_dram_parameter("wvT", [D, GD], bf16, isOutput=False)
    woT_e = nc.declare_dram_parameter("woT", [GD, D], bf16, isOutput=False)
    qnw_e = nc.declare_dram_parameter("qnw", [1, Dh], f32, isOutput=False)
    knw_e = nc.declare_dram_parameter("knw", [1, Dh], f32, isOutput=False)
    out_e = nc.declare_dram_parameter("out", [D, N], f32, isOutput=True)

    with nc.allow_low_precision(reason="bf16 PV+out path"), \
            tile.TileContext(nc) as tc:
        from contextlib import ExitStack

        with ExitStack() as ctx:
            ep = ctx.enter_context
            # distinct tile names are distinct tags; a tag gets `bufs` slots
            consts = ep(tc.tile_pool(name="consts", bufs=1))
            xpool = ep(tc.tile_pool(name="x", bufs=1))
            wqp = ep(tc.tile_pool(name="wq", bufs=1))
            wkp = ep(tc.tile_pool(name="wk", bufs=1))
            wvp = ep(tc.tile_pool(name="wv", bufs=1))
            wop = ep(tc.tile_pool(name="wo", bufs=1))
            vpool = ep(tc.tile_pool(name="v", bufs=1))
            qknp = ep(tc.tile_pool(name="qkn", bufs=1))  # 8 parity-named tags
            ptp = ep(tc.tile_pool(name="pt", bufs=2))
            otp = ep(tc.tile_pool(name="ot", bufs=1))
            scratch = ep(tc.tile_pool(name="scr", bufs=2))
            lslp = ep(tc.tile_pool(name="lsl", bufs=4))
            smallp = ep(tc.tile_pool(name="small", bufs=2))
            stp = ep(tc.tile_pool(name="st", bufs=2, space="PSUM"))
            pvp = ep(tc.tile_pool(name="pv", bufs=2, space="PSUM"))
            qkpp = ep(tc.tile_pool(name="qkp", bufs=1, space="PSUM"))
            auxp = ep(tc.tile_pool(name="aux", bufs=1, space="PSUM"))

            # ---- constants ----
            selq_raw = consts.tile([2, 128], f32)  # row g: qn_w at cols 64g..
            selk_raw = consts.tile([2, 128], f32)
            nc.vector.memset(selq_raw[:], 0.0)
            nc.vector.memset(selk_raw[:], 0.0)
            nc.sync.dma_start(selq_raw[0:1, 0:64], qnw_e[:, :])
            nc.sync.dma_start(selq_raw[1:2, 64:128], qnw_e[:, :])
            nc.sync.dma_start(selk_raw[0:1, 0:64], knw_e[:, :])
            nc.sync.dma_start(selk_raw[1:2, 64:128], knw_e[:, :])
            selq = consts.tile([2, 128], bf16)
            selk = consts.tile([2, 128], bf16)
            nc.vector.tensor_copy(selq[:], selq_raw[:])
            nc.vector.tensor_copy(selk[:], selk_raw[:])
            gones = consts.tile([128, 2], bf16)  # 64-group indicator
            nc.vector.memset(gones[:], 0.0)
            nc.vector.memset(gones[0:64, 0:1], 1.0)
            nc.vector.memset(gones[64:128, 1:2], 1.0)
            epsb = consts.tile([128, 1], f32)
            nc.vector.memset(epsb[:], EPS)

            # ---- activation / weight DMAs ----
            # x moves as whole [128, 2048] tiles (4KB rows sustain ~360GB/s
            # vs ~200 for 1KB rows), dt-order, on the Sync DMA queue; wv+wk
            # go out in parallel on the Scalar engine's DMA queue so the V
            # projection and the head K chunks aren't gated behind x.
            xt = [xpool.tile([128, N], bf16, name=f"xt{i}") for i in range(DT)]
            for i in range(DT):
                nc.sync.dma_start(xt[i][:], xT_e[128 * i : 128 * (i + 1), :])
            wdge2 = nc.scalar if hasattr(nc.scalar, "dma_start") else nc.sync
            wv = []
            for i in range(DT):
                t = wvp.tile([128, GD], bf16, name=f"wv{i}")
                wdge2.dma_start(t[:], wvT_e[128 * i : 128 * (i + 1), :])
                wv.append(t)
            wk, wq = [], []
            for i in range(DT):
                t = wkp.tile([128, GD], bf16, name=f"wk{i}")
                wdge2.dma_start(t[:], wkT_e[128 * i : 128 * (i + 1), :])
                wk.append(t)
            for i in range(DT):
                t = wqp.tile([128, GD], bf16, name=f"wq{i}")
                nc.sync.dma_start(t[:], wqT_e[128 * i : 128 * (i + 1), :])
                wq.append(t)
            wo = []
            for i in range(KT):
                t = wop.tile([128, D], bf16, name=f"wo{i}")
                nc.sync.dma_start(t[:], woT_e[128 * i : 128 * (i + 1), :])
                wo.append(t)

            # ---- V projection -> V_aug bf16 [t-tile][128, HPC, Dh+1] ----
            # dt-outer over all 8 PSUM banks (8 token tiles at once): the
            # first chain needs only xt[0]+wv[0], so PE starts ~11us in,
            # paced by the x DMA stream instead of waiting for all of x.
            vsb = [None] * NT

            def v_pass(tts, accs):
                for dt_ in range(DT):
                    for q, tt in enumerate(tts):
                        nc.tensor.matmul(
                            accs[q],
                            xt[dt_][:, 128 * tt : 128 * (tt + 1)],
                            wv[dt_][:],
                            start=(dt_ == 0),
                            stop=(dt_ == DT - 1),
                        )
                for q, tt in enumerate(tts):
                    vt = vpool.tile([128, HPC, Dh + 1], bf16, name=f"v{tt}")
                    nc.vector.tensor_copy(
                        vt[:, :, 0:Dh], accs[q].rearrange("p (h d) -> p h d", h=HPC)
                    )
                    nc.vector.memset(vt[:, :, Dh : Dh + 1], 1.0)
                    vsb[tt] = vt

            a0 = stp.tile([128, 1024], f32, name="st0", bufs=1)
            a1 = stp.tile([128, 1024], f32, name="st1", bufs=1)
            v_pass(
                range(8),
                [
                    a0[:, 0:512],
                    a0[:, 512:1024],
                    a1[:, 0:512],
                    a1[:, 512:1024],
                    pvp.tile([128, 512], f32, name="pv"),
                    pvp.tile([128, 512], f32, name="pv"),
                    qkpp.tile([128, 512], f32, name="qkp"),
                    auxp.tile([128, 512], f32, name="aux"),
                ],
            )

            # ---- O.T accumulator tiles (row block hp, all heads) ----
            ot = [otp.tile([128, N], bf16, name=f"ot{i}") for i in range(KT)]

            qkn_all = [
                [qknp.tile([128, N], bf16, name=f"qkn{hp}_{side}") for side in range(2)]
                for hp in range(4)
            ]

            def qk_chunk(hp, side, c4, pool, tag):
                """(emit_a, emit_b) projecting + RMS-normalizing one
                512-token chunk of one side of head pair hp, accumulating
                through PSUM pool `pool` tag `tag`."""
                wmat, sel = ((wq, selq), (wk, selk))[side]
                dst = qkn_all[hp][side]
                cell = []  # lsl tile, created at emit_a time (emission order)

                def emit_a():
                    qps = pool.tile([128, 512], f32, name=tag, bufs=1)
                    for dt_ in range(DT):
                        nc.tensor.matmul(
                            qps[:],
                            wmat[dt_][:, 128 * hp : 128 * (hp + 1)],
                            xt[dt_][:, 512 * c4 : 512 * (c4 + 1)],
                            start=(dt_ == 0),
                            stop=(dt_ == DT - 1),
                        )
                    sl = dst[:, 512 * c4 : 512 * (c4 + 1)]
                    nc.vector.tensor_copy(sl, qps[:])
                    q2 = scratch.tile([128, 512], bf16, name="q2")
                    nc.vector.tensor_mul(q2[:], sl, sl)
                    # ms[g, t] = sum of squares within each 64-row head
                    msps = pool.tile([128, 512], f32, name=tag, bufs=1)
                    nc.tensor.matmul(
                        msps[0:2, :], gones[:], q2[:], start=True, stop=True
                    )
                    lsl = lslp.tile([2, 512], f32, name="lsl")
                    cell.append(lsl)
                    nc.scalar.activation(
                        lsl[:], msps[0:2, :], AF.Ln, bias=epsb[0:2], scale=1.0 / Dh
                    )

                def emit_b():
                    # rinv = exp(-0.5*ln), expand with qn_w folded into sel
                    rinv = scratch.tile([2, 512], bf16, name="rinv", bufs=4)
                    nc.scalar.activation(rinv[:], cell[0][:], AF.Exp, scale=-0.5)
                    rexp = pool.tile([128, 512], f32, name=tag, bufs=1)
                    nc.tensor.matmul(rexp[:], sel[:], rinv[:], start=True, stop=True)
                    sl = dst[:, 512 * c4 : 512 * (c4 + 1)]
                    nc.vector.tensor_mul(sl, sl, rexp[:])

                return emit_a, emit_b

            # ---- head: K(hp0) + Q(hp0, chunk 0), 2-bank pipeline; the V
            # projection's second half is emitted AFTER the chunks on the
            # other four banks, so the scheduler fills the chunk chains'
            # ScalarE-latency stalls with V matmuls
            head_specs = [(0, 1, c4) for c4 in range(4)] + [(0, 0, 0)]
            head_banks = [(stp, "st0"), (stp, "st1")]
            head_ab = [
                qk_chunk(hp, side, c4, *head_banks[i % 2])
                for i, (hp, side, c4) in enumerate(head_specs)
            ]
            emitted = []
            for i in range(len(head_ab)):
                emitted.append(head_ab[i][0])
                if i >= 1:
                    emitted.append(head_ab[i - 1][1])
            emitted.append(head_ab[-1][1])
            for em in emitted:
                em()
            for sub in range(2):
                v_pass(
                    range(8 + 4 * sub, 12 + 4 * sub),
                    [
                        pvp.tile([128, 512], f32, name="pv"),
                        pvp.tile([128, 512], f32, name="pv"),
                        qkpp.tile([128, 512], f32, name="qkp"),
                        auxp.tile([128, 512], f32, name="aux"),
                    ],
                )

            # ---- filler assignment for the master loop ----
            # blocks B = 4*qc + hp, steps k = 16*B + j
            fillers = {}

            def add_filler(k, fn):
                fillers.setdefault(k, []).append(fn)

            banks = [(qkpp, "qkp"), (auxp, "aux")]

            def add_chunk(k, hp, side, c4, bank, bgap=3):
                a, b = qk_chunk(hp, side, c4, *banks[bank])
                add_filler(k, a)
                add_filler(k + bgap, b)

            # sweep-0 carries the remaining K (+ first Q) projections,
            # balanced ~4-5 chunks per block within each piece's deadline
            # (K(hp,c4) is first read at step 16*hp + 4*c4)
            for c4 in range(4):
                add_chunk(3 * c4, 1, 1, c4, c4 % 2)
            add_chunk(10, 1, 0, 0, 0)
            add_chunk(16, 2, 1, 0, 0)
            add_chunk(20, 2, 1, 1, 1)
            add_chunk(24, 2, 0, 0, 0)
            add_chunk(28, 2, 1, 2, 1)
            add_chunk(32, 2, 1, 3, 0)
            add_chunk(36, 3, 1, 0, 1)
            add_chunk(40, 3, 0, 0, 0)
            add_chunk(44, 3, 1, 1, 1)
            add_chunk(48, 3, 1, 2, 0)
            add_chunk(52, 3, 1, 3, 1)
            # Q chunks for sweeps >= 1: block 3 preloads blocks 4-5; block
            # B >= 4 emits the Q chunk for block B+2, its rinv spread away
            # from its ln so the two ScalarE displacements don't stack
            add_chunk(56, 0, 0, 1, 0)
            add_chunk(60, 1, 0, 1, 1)
            for Bs in range(4, 14):
                hp2_, qc2_ = (Bs + 2) % 4, (Bs + 2) // 4
                add_chunk(16 * Bs + 2, hp2_, 0, qc2_, 0, bgap=6)

            pending = []  # deferred normalize tails (DVE muls)

            def outproj(tch, dos, bank=0):
                for do in dos:
                    pool, tag = banks[bank]
                    bank ^= 1
                    yps = pool.tile([128, 512], f32, name=tag)
                    for kt_ in range(KT):
                        nc.tensor.matmul(
                            yps[:],
                            wo[kt_][:, 128 * do : 128 * (do + 1)],
                            ot[kt_][:, 512 * tch : 512 * (tch + 1)],
                            start=(kt_ == 0),
                            stop=(kt_ == KT - 1),
                        )
                    ysb = scratch.tile([128, 512], f32, name="q2")
                    nc.vector.tensor_copy(ysb[:], yps[:])
                    nc.sync.dma_start(
                        out_e[128 * do : 128 * (do + 1), 512 * tch : 512 * (tch + 1)],
                        ysb[:],
                    )

            # flat 2-deep pipeline over all (block, k-tile) steps:
            # per step k emit S.T(k), exp(k-1), PV(k-2)
            steps = [
                (qc, hp, j) for qc in range(4) for hp in range(4)
                for j in range(NT)
            ]
            n = len(steps)
            blk_pvs = {}
            sts = {}
            pts = {}

            def emit_st(k):
                qc, hp, j = steps[k]
                qn, kn = qkn_all[hp]
                st = stp.tile([128, 1024], f32, name=f"st{k % 2}", bufs=1)
                for side in range(2):
                    p0 = 64 * side
                    nc.tensor.matmul(
                        st[:, 512 * side : 512 * (side + 1)],
                        kn[p0 : p0 + 64, 128 * j : 128 * (j + 1)],
                        qn[p0 : p0 + 64, 512 * qc : 512 * (qc + 1)],
                        start=True,
                        stop=True,
                    )
                sts[k] = st

            def emit_exp(k):
                pt = ptp.tile([128, 1024], bf16, name=f"pt{k % 6}", bufs=1)
                nc.scalar.activation(pt[:], sts.pop(k)[:], AF.Exp, scale=Dh**-0.5)
                pts[k] = pt

            def emit_pv(k):
                qc, hp, j = steps[k]
                if j == 0:
                    blk_pvs[(hp, qc)] = [
                        pvp.tile([Dh + 1, 512], f32, name="pv") for _ in range(2)
                    ]
                pvs = blk_pvs[(hp, qc)]
                pt = pts.pop(k)
                for side in range(2):
                    nc.tensor.matmul(
                        pvs[side][:],
                        vsb[j][:, 2 * hp + side, :],
                        pt[:, 512 * side : 512 * (side + 1)],
                        start=(j == 0),
                        stop=(j == NT - 1),
                    )
                if j == NT - 1:
                    drain_block(pvs, hp, qc)
                if j == 2 and pending:
                    # normalize tails for the block drained ~2 blocks ago:
                    # must precede this sweep's outproj reads of O.T (j>=5)
                    for fn in pending:
                        fn()
                    pending.clear()
                # previous token-chunk's output projection, two columns per
                # block, spread over the whole following sweep
                if qc > 0 and j in (5, 11):
                    do0 = 2 * hp + (0 if j == 5 else 1)
                    outproj(qc - 1, dos=(do0,), bank=1)

            def drain_block(pvs, hp, qc):
                last = hp == 3 and qc == 3
                for side in range(2):
                    p0 = 64 * side
                    if last:
                        # nothing needs the pv slots after the final block:
                        # normalize straight out of PSUM, skip the copy
                        oraw = pvs[side]
                    else:
                        # free the pv slot quickly; normalize out of scratch
                        oraw = scratch.tile([Dh + 1, 512], f32, name="oraw", bufs=4)
                        nc.vector.tensor_copy(oraw[:], pvs[side][:])
                    den0 = smallp.tile([1, 512], f32, name="den0", bufs=4)
                    nc.vector.tensor_copy(den0[:], pvs[side][Dh : Dh + 1, :])
                    rdenf = smallp.tile([1, 512], f32, name="rdenf", bufs=4)
                    nc.vector.reciprocal_approx_fast(rdenf[:], den0[:])
                    # expand 1/den across the 64 dh rows on the idle GpSimd
                    rde = scratch.tile([Dh, 512], f32, name="rde", bufs=4)
                    nc.gpsimd.partition_broadcast(rde[:], rdenf[:], channels=Dh)
                    osl = ot[hp][p0 : p0 + 64, 512 * qc : 512 * (qc + 1)]

                    def fin(oraw=oraw, rde=rde, osl=osl):
                        nc.vector.tensor_mul(osl, oraw[0:Dh, :], rde[:])

                    pending.append(fin)

            for k in range(n + 2):
                if k < n:
                    emit_st(k)
                if 0 < k <= n:
                    emit_exp(k - 1)
                if 1 < k <= n + 1:
                    emit_pv(k - 2)
                for fn in fillers.pop(k, []):
                    fn()
            # tail: pre-accumulate Wo columns 0/1 over the three finished
            # O.T row blocks while the last block's normalize chain drains
            parts = []
            for do in range(2):
                yps = banks[do][0].tile([128, 512], f32, name=banks[do][1])
                for kt_ in range(KT - 1):
                    nc.tensor.matmul(
                        yps[:],
                        wo[kt_][:, 128 * do : 128 * (do + 1)],
                        ot[kt_][:, 512 * 3 : 512 * 4],
                        start=(kt_ == 0),
                        stop=False,
                    )
                parts.append(yps)
            for fn in pending:
                fn()
            pending.clear()
            for do in range(2):
                nc.tensor.matmul(
                    parts[do][:],
                    wo[KT - 1][:, 128 * do : 128 * (do + 1)],
                    ot[KT - 1][:, 512 * 3 : 512 * 4],
                    start=False,
                    stop=True,
                )
                ysb = scratch.tile([128, 512], f32, name="q2")
                nc.vector.tensor_copy(ysb[:], parts[do][:])
                nc.sync.dma_start(
                    out_e[128 * do : 128 * (do + 1), 512 * 3 : 512 * 4],
                    ysb[:],
                )
            outproj(3, range(2, DT))

    nc.compile()
    return nc


def _get_nc():
    if "nc" not in _NC_CACHE:
        _NC_CACHE["nc"] = _build_nc()
    return _NC_CACHE["nc"]


def make_in_maps(x, Wq, Wk, Wv, Wo, qn_w, kn_w):
    x = np.asarray(x, np.float32)
    Wq, Wk, Wv, Wo = (np.asarray(w, np.float32) for w in (Wq, Wk, Wv, Wo))
    qn_w = np.asarray(qn_w, np.float32).reshape(1, Dh)
    kn_w = np.asarray(kn_w, np.float32).reshape(1, Dh)
    in_maps = []
    for c in range(8):
        b, g = c // 2, c % 2
        sl = slice(GD * g, GD * (g + 1))
        in_maps.append(
            {
                "xT": np.ascontiguousarray(x[b].T).astype(ml_dtypes.bfloat16),
                "wqT": np.ascontiguousarray(Wq[sl, :].T).astype(ml_dtypes.bfloat16),
                "wkT": np.ascontiguousarray(Wk[sl, :].T).astype(ml_dtypes.bfloat16),
                "wvT": np.ascontiguousarray(Wv[sl, :].T).astype(ml_dtypes.bfloat16),
                "woT": np.ascontiguousarray(Wo[:, sl].T).astype(ml_dtypes.bfloat16),
                "qnw": qn_w,
                "knw": kn_w,
            }
        )
    return in_maps


def assemble(results):
    out = np.empty((B, N, D), np.float32)
    for b in range(B):
        out[b] = (
            results[2 * b]["out"].astype(np.float32)
            + results[2 * b + 1]["out"].astype(np.float32)
        ).T
    return out


def kernel(x, Wq, Wk, Wv, Wo, qn_w, kn_w):
    from concourse.bass_utils import run_bass_kernel_spmd

    nc = _get_nc()
    in_maps = make_in_maps(x, Wq, Wk, Wv, Wo, qn_w, kn_w)
    res = run_bass_kernel_spmd(nc, in_maps, core_ids=list(range(8)))
    return assemble(res.results)
